# revision 15
# baseline (speedup 1.0000x reference)
"""Causal self-attention (B=2, T=2048, C=1024, H=16) on 8 TRN2 NeuronCores.

Sharding: 8 cores = 2 batches x 4 head-groups (4 heads each).
Each core computes qkv projection for its heads, attention, and a partial
output projection (its rows of w_proj); the host sums the 4 partials per
batch and adds b_proj.

v2: single software-pipelined loop over 512-token chunks.  Per chunk:
QKV projection + rope, V projection, output projection of the PREVIOUS
chunk, then attention for the chunk (both head-pairs).  This keeps the
PE fed continuously instead of the v1 phase-sequential structure.

Device-side layout choices:
  - all matmuls in bf16 (full PE rate, halves DMA); f32 psum accumulate.
  - x is fed transposed (xT [C, T], fully SBUF-resident) so the
    contraction dim C sits on SBUF partitions for the projections.
  - q, k are produced transposed ([d, t], head pairs stacked on
    partitions) so scores are computed TRANSPOSED: S^T[j, i] = kT.T @ qT.
    Softmax over j (partitions) needs no reductions: the AV matmul with a
    ones-augmented V yields the denominators as an extra output row.
  - v is produced natural ([t, d]) via a second projection.
  - softmax exp runs in fp32 on the scalar engine (scale=1/8 folded in;
    scores are O(+-10) so exp cannot overflow), output cast to bf16.
  - causal mask: static 0/1 bf16 masks multiplied on DVE (diagonal j-block
    tiles only); rope swap-strips run on the Pool engine, the rest of rope
    on DVE.  The scalar engine does nothing but exp.
"""

import sys
import os

for _p in ("/opt/trn_rl_repo", "/root/.axon_site/_ro/trn_rl_repo"):
    if os.path.isdir(_p) and _p not in sys.path:
        sys.path.insert(0, _p)

import numpy as np
import concourse.bass as bass
import concourse.mybir as mybir
import concourse.tile as tile
from concourse import bacc
from concourse.bass_utils import run_bass_kernel_spmd

B, T, C, H = 2, 2048, 1024, 16
HS = C // H          # 64
HALF = HS // 2       # 32
NCORES = 8
NH = 4               # heads per core
TCH = 512            # chunk: t-columns per pipeline stage
NCH = T // TCH       # 4 chunks
CB = C // 128        # 8 contraction blocks
NTB = T // 128       # 16 t/j blocks
F32 = mybir.dt.float32
BF16 = mybir.dt.bfloat16
MMD = BF16
AF = mybir.ActivationFunctionType
ALU = mybir.AluOpType

_CACHED = {}


def _build_nc():
    nc = bacc.Bacc("TRN2", target_bir_lowering=False, debug=False)

    xt = nc.dram_tensor("xt", [C, T], MMD, kind="ExternalInput").ap()
    wqk = nc.dram_tensor("wqk", [C, 512], MMD, kind="ExternalInput").ap()
    wv = nc.dram_tensor("wv", [C, 256], MMD, kind="ExternalInput").ap()
    wproj = nc.dram_tensor("wproj", [256, C], MMD, kind="ExternalInput").ap()
    bqk = nc.dram_tensor("bqk", [4, 128], F32, kind="ExternalInput").ap()
    cosrep = nc.dram_tensor("cosrep", [128, T], F32, kind="ExternalInput").ap()
    sinsw = nc.dram_tensor("sinsw", [128, T], F32, kind="ExternalInput").ap()
    ones64 = nc.dram_tensor("ones64", [128, 64], F32, kind="ExternalInput").ap()
    yout = nc.dram_tensor("yout", [T, C], F32, kind="ExternalOutput").ap()

    with tile.TileContext(nc) as tc:
        with (
            tc.tile_pool(name="const", bufs=1) as const,
            tc.tile_pool(name="persist", bufs=1) as persist,
            tc.tile_pool(name="work", bufs=2) as work,
            tc.tile_pool(name="attnp", bufs=6) as attnp,
            tc.tile_pool(name="ps", bufs=1, space="PSUM") as ps,
        ):
            # ---- constant + input loads ----------------------------------------
            # scalar queue: wqk blocks first (first matmul needs cb0), then
            # cos/sin (rope of chunk 0), wv (V of chunk 0), wproj (proj, late).
            wqk_sb = const.tile([128, CB * 512], MMD)
            for cb in range(CB):
                nc.scalar.dma_start(
                    out=wqk_sb[:, cb * 512 : (cb + 1) * 512],
                    in_=wqk[cb * 128 : (cb + 1) * 128, :],
                )
            cos_sb = const.tile([128, T], F32)
            nc.scalar.dma_start(out=cos_sb, in_=cosrep)
            sin_sb = const.tile([128, T], F32)
            nc.scalar.dma_start(out=sin_sb, in_=sinsw)
            wv_sb = const.tile([128, CB * 256], MMD)
            nc.scalar.dma_start(
                out=wv_sb.rearrange("p (cb m) -> p cb m", cb=CB),
                in_=wv.rearrange("(cb p) m -> p cb m", p=128),
            )
            wproj_sb = const.tile([128, 2 * C], MMD)
            nc.scalar.dma_start(
                out=wproj_sb.rearrange("p (cb n) -> p cb n", cb=2),
                in_=wproj.rearrange("(cb p) n -> p cb n", p=128),
            )

            # sync queue: tiny consts, xt chunk 0, masks, xt chunks 1-3.
            bqk_sb = const.tile([128, 4], F32)
            for mt in range(4):
                nc.sync.dma_start(out=bqk_sb[:, mt : mt + 1], in_=bqk[mt, :][:, None])
            ones_sb = const.tile([128, 64], F32)
            nc.sync.dma_start(out=ones_sb, in_=ones64)
            xt_sb = persist.tile([128, CB * T], MMD)   # [c128, cb, t]
            for cb in range(CB):
                nc.sync.dma_start(
                    out=xt_sb[:, cb * T : cb * T + TCH],
                    in_=xt[cb * 128 : (cb + 1) * 128, 0:TCH],
                )
            for cb in range(CB):
                nc.sync.dma_start(
                    out=xt_sb[:, cb * T + TCH : (cb + 1) * T],
                    in_=xt[cb * 128 : (cb + 1) * 128, TCH:T],
                )

            # ---- persistent intermediates --------------------------------------
            qt_sb = persist.tile([128, 2 * T], MMD)   # [Q01 | Q23], [d(2 heads), t]
            kt_sb = persist.tile([128, 2 * T], MMD)
            v_sb = persist.tile([128, NTB * 260], MMD)  # per j-block: 4x(64 v + 1 one)
            ctx0 = persist.tile([128, T], MMD)        # heads 0,1 ctxT
            ctx1 = persist.tile([128, T], MMD)        # heads 2,3 ctxT

            # ones columns of v_sb (once)
            nc.gpsimd.tensor_copy(
                v_sb.rearrange("p (tb h d) -> p tb h d", tb=NTB, h=4)[:, :, :, 64:65],
                ones_sb.rearrange("p (a b c) -> p a b c", a=NTB, b=4),
            )

            def emit_proj(tci):
                # output projection for chunk tci (ctx columns ready)
                for sub in range(4):
                    tb = tci * 4 + sub
                    yp = ps.tile([128, 1024], F32, tag="pa", bufs=3, name=f"yp{tb}")
                    for ncol in range(2):
                        for cbb in range(2):
                            ctx_t = ctx0 if cbb == 0 else ctx1
                            nc.tensor.matmul(
                                yp[:, ncol * 512 : (ncol + 1) * 512],
                                lhsT=ctx_t[:, tb * 128 : (tb + 1) * 128],
                                rhs=wproj_sb[:, cbb * C + ncol * 512 : cbb * C + (ncol + 1) * 512],
                                start=(cbb == 0),
                                stop=(cbb == 1),
                            )
                    ysb = work.tile([128, 1024], F32, tag="ysb", bufs=2, name=f"ysb{tb}")
                    nc.vector.tensor_copy(ysb, yp)
                    nc.sync.dma_start(out=yout[tb * 128 : (tb + 1) * 128, :], in_=ysb)

            for tci in range(NCH):
                tsl = slice(tci * TCH, (tci + 1) * TCH)

                # ---- QKV projection + rope (M-tiles: Q01 Q23 K01 K23) ----------
                for mt in range(4):
                    pq = ps.tile([128, 1024], F32, tag="pa", bufs=3, name=f"pq{tci}_{mt}")
                    for cb in range(CB):
                        nc.tensor.matmul(
                            pq[:, 0:TCH],
                            lhsT=wqk_sb[:, cb * 512 + mt * 128 : cb * 512 + (mt + 1) * 128],
                            rhs=xt_sb[:, cb * T + tci * TCH : cb * T + (tci + 1) * TCH],
                            start=(cb == 0),
                            stop=(cb == CB - 1),
                        )
                    # rope: out = (pq+b)*cos + swap(pq+b)*sin.  swap-muls read
                    # qb/sin from SBUF so they run on the Pool engine.
                    m1 = work.tile([128, TCH], F32, tag="m1", bufs=2, name=f"m1_{tci}_{mt}")
                    nc.vector.scalar_tensor_tensor(
                        out=m1, in0=pq[:, 0:TCH], scalar=bqk_sb[:, mt : mt + 1],
                        in1=cos_sb[:, tsl], op0=ALU.add, op1=ALU.mult,
                    )
                    qb = work.tile([128, TCH], F32, tag="qb", bufs=2, name=f"qb{tci}_{mt}")
                    nc.scalar.activation(
                        qb, pq[:, 0:TCH], AF.Identity, bias=bqk_sb[:, mt : mt + 1], scale=1.0
                    )
                    swp = work.tile([128, TCH], F32, tag="swp", bufs=2, name=f"swp{tci}_{mt}")
                    for dst0, src0 in ((0, 32), (32, 0), (64, 96), (96, 64)):
                        nc.gpsimd.tensor_mul(
                            swp[dst0 : dst0 + 32, :],
                            qb[src0 : src0 + 32, :],
                            sin_sb[src0 : src0 + 32, tsl],
                        )
                    dest = qt_sb if mt < 2 else kt_sb
                    dcol = (mt % 2) * T + tci * TCH
                    nc.vector.tensor_add(dest[:, dcol : dcol + TCH], m1, swp)

                # ---- V projection: natural layout [t, d], 4 t-blocks -----------
                pv = ps.tile([128, 1024], F32, tag="pa", bufs=3, name=f"pv{tci}")
                for sub in range(4):
                    tb = tci * 4 + sub
                    for cb in range(CB):
                        nc.tensor.matmul(
                            pv[:, sub * 256 : (sub + 1) * 256],
                            lhsT=xt_sb[:, cb * T + tb * 128 : cb * T + (tb + 1) * 128],
                            rhs=wv_sb[:, cb * 256 : (cb + 1) * 256],
                            start=(cb == 0),
                            stop=(cb == CB - 1),
                        )
                for sub in range(4):
                    tb = tci * 4 + sub
                    vdst = v_sb[:, tb * 260 : tb * 260 + 260].rearrange(
                        "p (h d) -> p h d", h=4
                    )[:, :, 0:64]
                    vsrc = pv[:, sub * 256 : (sub + 1) * 256].rearrange(
                        "p (h d) -> p h d", h=4
                    )
                    if tci < 2:
                        nc.scalar.copy(vdst, vsrc)
                    else:
                        nc.vector.tensor_copy(vdst, vsrc)

                # ---- output projection of the previous chunk -------------------
                if tci > 0:
                    emit_proj(tci - 1)

                # ---- attention for i-chunk tci (head-pair inner) ---------------
                njb = 4 * (tci + 1)
                for pair in range(2):
                    qt_p = qt_sb[:, pair * T : (pair + 1) * T]
                    kt_p = kt_sb[:, pair * T : (pair + 1) * T]
                    ctx_p = ctx0 if pair == 0 else ctx1
                    ctxps = [
                        ps.tile([65, 512], F32, tag="ctx", bufs=2, name=f"ctxp{pair}_{tci}_{hh}")
                        for hh in range(2)
                    ]

                    def emit_av(at_pair, duo):
                        for hh in range(2):
                            for half in range(2):
                                jb = duo * 2 + half
                                h_loc = pair * 2 + hh
                                nc.tensor.matmul(
                                    ctxps[hh],
                                    lhsT=v_sb[:, jb * 260 + h_loc * 65 : jb * 260 + (h_loc + 1) * 65],
                                    rhs=at_pair[hh][:, half * 512 : (half + 1) * 512],
                                    start=(jb == 0),
                                    stop=(jb == njb - 1),
                                )

                    pending = []
                    for duo in range(njb // 2):
                        st = [
                            ps.tile([128, 1024], F32, tag="pa", bufs=3, name=f"st{pair}_{tci}_{duo}_{hh}")
                            for hh in range(2)
                        ]
                        # interleave the two heads' QK matmuls (disjoint row
                        # strips 0-63 / 64-127)
                        for half in range(2):
                            jb = duo * 2 + half
                            for hh in range(2):
                                nc.tensor.matmul(
                                    st[hh][:, half * 512 : (half + 1) * 512],
                                    lhsT=kt_p[hh * 64 : (hh + 1) * 64, jb * 128 : (jb + 1) * 128],
                                    rhs=qt_p[hh * 64 : (hh + 1) * 64, tsl],
                                    start=True,
                                    stop=True,
                                )
                        if len(pending) >= 2:
                            emit_av(*pending.pop(0))
                        at_pair = []
                        for hh in range(2):
                            at = attnp.tile([128, 1024], MMD, tag="attn", bufs=6, name=f"at{pair}_{tci}_{duo}_{hh}")
                            nc.scalar.activation(at, st[hh], AF.Exp, scale=0.125)
                            if duo >= 2 * tci:  # diagonal duo: zero j > i
                                nc.gpsimd.affine_select(
                                    out=at,
                                    in_=at,
                                    compare_op=ALU.is_ge,
                                    fill=0.0,
                                    base=tci * TCH - duo * 2 * 128,
                                    channel_multiplier=-1,
                                    pattern=[[-128, 2], [1, 512]],
                                )
                            at_pair.append(at)
                        pending.append((at_pair, duo))
                    for p in pending:
                        emit_av(*p)

                    # normalize: ctx[d, i] /= denom[i] (denom = row 64)
                    for hh in range(2):
                        ctxu = work.tile([64, 512], F32, tag="ctxu", bufs=2, name=f"cu{pair}_{tci}_{hh}")
                        nc.vector.tensor_copy(ctxu, ctxps[hh][0:64, :])
                        dn = work.tile([1, 512], F32, tag="dnrow", bufs=2, name=f"dn{pair}_{tci}_{hh}")
                        nc.vector.tensor_copy(dn, ctxps[hh][64:65, :])
                        rc = work.tile([1, 512], F32, tag="recip", bufs=2, name=f"rc{pair}_{tci}_{hh}")
                        nc.vector.reciprocal_approx_fast(out=rc, in_=dn)
                        bcast = work.tile([64, 512], F32, tag="bcast", bufs=2, name=f"bcast{pair}_{tci}_{hh}")
                        nc.gpsimd.partition_broadcast(bcast, rc)
                        nc.vector.tensor_mul(
                            ctx_p[hh * 64 : (hh + 1) * 64, tsl],
                            ctxu,
                            bcast,
                        )

            emit_proj(NCH - 1)

    nc.compile()
    return nc


def _prep_core_inputs(x, cos, sin, w_attn, b_attn, w_proj):
    """Build the 8 per-core input maps (host-side shard/reorder)."""
    import ml_dtypes
    mmnp = ml_dtypes.bfloat16
    x = np.asarray(x, dtype=np.float32)
    cos = np.asarray(cos, dtype=np.float32).reshape(T, HALF)
    sin = np.asarray(sin, dtype=np.float32).reshape(T, HALF)
    w_attn = np.asarray(w_attn, dtype=np.float32)
    b_attn = np.asarray(b_attn, dtype=np.float32)
    w_proj = np.asarray(w_proj, dtype=np.float32)

    cosT = np.ascontiguousarray(cos.T)               # [32, T]
    sinT = np.ascontiguousarray(sin.T)
    cosrep = np.tile(cosT, (4, 1))                   # [128, T]
    sin_sw = np.concatenate([sinT, -sinT, sinT, -sinT], axis=0)
    ones64 = np.ones((128, 64), np.float32)

    xts = [np.ascontiguousarray(x[b].T).astype(mmnp) for b in range(B)]  # [C, T] each

    in_maps = []
    for core in range(NCORES):
        b = core // 4
        g = core % 4
        heads = [4 * g + i for i in range(NH)]
        # q/k column blocks: M-tiles [Q(h0,h1), Q(h2,h3), K(h0,h1), K(h2,h3)]
        qcols, bq = [], []
        for mt, (base, hs) in enumerate(
            [(0, heads[0:2]), (0, heads[2:4]), (C, heads[0:2]), (C, heads[2:4])]
        ):
            cols = np.concatenate([np.arange(base + h * HS, base + (h + 1) * HS) for h in hs])
            qcols.append(cols)
            bq.append(b_attn[cols])
        wqk_c = np.ascontiguousarray(w_attn[:, np.concatenate(qcols)]).astype(mmnp)
        bqk_c = np.stack(bq)                                            # [4, 128]
        vcols = np.concatenate(
            [np.arange(2 * C + h * HS, 2 * C + (h + 1) * HS) for h in heads]
        )
        wv_c = np.ascontiguousarray(w_attn[:, vcols]).astype(mmnp)
        wproj_c = np.ascontiguousarray(w_proj[g * 256 : (g + 1) * 256, :]).astype(mmnp)
        in_maps.append(
            {
                "xt": xts[b],
                "wqk": wqk_c,
                "wv": wv_c,
                "wproj": wproj_c,
                "bqk": np.ascontiguousarray(bqk_c),
                "cosrep": np.ascontiguousarray(cosrep),
                "sinsw": np.ascontiguousarray(sin_sw),
                "ones64": ones64,
            }
        )
    return in_maps


def kernel(x, cos, sin, w_attn, b_attn, w_proj, b_proj, _want_trace=False):
    if "nc" not in _CACHED:
        _CACHED["nc"] = _build_nc()
    nc = _CACHED["nc"]
    in_maps = _prep_core_inputs(x, cos, sin, w_attn, b_attn, w_proj)
    res = run_bass_kernel_spmd(
        nc, in_maps, core_ids=list(range(NCORES)), trace=_want_trace
    )
    _CACHED["last_result"] = res
    b_proj = np.asarray(b_proj, dtype=np.float32)
    # v-bias folds out of attention (softmax rows sum to 1): it contributes a
    # constant b_v @ w_proj to every output row, added here with b_proj.
    bv = np.asarray(b_attn, dtype=np.float32)[2 * C : 3 * C]
    bias_full = b_proj + bv @ np.asarray(w_proj, dtype=np.float32)
    out = np.empty((B, T, C), np.float32)
    for b in range(B):
        acc = res.results[b * 4]["yout"].astype(np.float32).copy()
        for g in range(1, 4):
            acc += res.results[b * 4 + g]["yout"]
        out[b] = acc + bias_full[None, :]
    return out


# revision 20
# speedup vs baseline: 1.3705x; 1.3705x over previous
"""Causal self-attention (B=2, T=2048, C=1024, H=16) on 8 TRN2 NeuronCores.

Sharding: 8 cores = 2 batches x 4 head-groups (4 heads each).
Each core computes qkv projection for its heads, attention, and a partial
output projection (its rows of w_proj); the host sums the 4 partials per
batch and adds b_proj.

v2: single software-pipelined loop over 512-token chunks.  Per chunk:
QKV projection + rope, V projection, output projection of the PREVIOUS
chunk, then attention for the chunk (both head-pairs).  This keeps the
PE fed continuously instead of the v1 phase-sequential structure.

Device-side layout choices:
  - all matmuls in bf16 (full PE rate, halves DMA); f32 psum accumulate.
  - x is fed transposed (xT [C, T], fully SBUF-resident) so the
    contraction dim C sits on SBUF partitions for the projections.
  - q, k are produced transposed ([d, t], head pairs stacked on
    partitions) so scores are computed TRANSPOSED: S^T[j, i] = kT.T @ qT.
    Softmax over j (partitions) needs no reductions: the AV matmul with a
    ones-augmented V yields the denominators as an extra output row.
  - v is produced natural ([t, d]) via a second projection.
  - softmax exp runs in fp32 on the scalar engine (scale=1/8 folded in;
    scores are O(+-10) so exp cannot overflow), output cast to bf16.
  - causal mask: static 0/1 bf16 masks multiplied on DVE (diagonal j-block
    tiles only); rope swap-strips run on the Pool engine, the rest of rope
    on DVE.  The scalar engine does nothing but exp.
"""

import sys
import os

for _p in ("/opt/trn_rl_repo", "/root/.axon_site/_ro/trn_rl_repo"):
    if os.path.isdir(_p) and _p not in sys.path:
        sys.path.insert(0, _p)

import numpy as np
import concourse.bass as bass
import concourse.mybir as mybir
import concourse.tile as tile
from concourse import bacc
from concourse.bass_utils import run_bass_kernel_spmd

B, T, C, H = 2, 2048, 1024, 16
HS = C // H          # 64
HALF = HS // 2       # 32
NCORES = 8
NH = 4               # heads per core
TCH = 512            # chunk: t-columns per pipeline stage
NCH = T // TCH       # 4 chunks
CB = C // 128        # 8 contraction blocks
NTB = T // 128       # 16 t/j blocks
F32 = mybir.dt.float32
BF16 = mybir.dt.bfloat16
MMD = BF16
AF = mybir.ActivationFunctionType
ALU = mybir.AluOpType

_CACHED = {}


def _build_nc():
    nc = bacc.Bacc("TRN2", target_bir_lowering=False, debug=False)

    xt = nc.dram_tensor("xt", [C, T], MMD, kind="ExternalInput").ap()
    wqk = nc.dram_tensor("wqk", [C, 512], MMD, kind="ExternalInput").ap()
    wv = nc.dram_tensor("wv", [C, 256], MMD, kind="ExternalInput").ap()
    wproj = nc.dram_tensor("wproj", [256, C], MMD, kind="ExternalInput").ap()
    bqk = nc.dram_tensor("bqk", [4, 128], F32, kind="ExternalInput").ap()
    cosrep = nc.dram_tensor("cosrep", [128, T], F32, kind="ExternalInput").ap()
    sinsw = nc.dram_tensor("sinsw", [128, T], F32, kind="ExternalInput").ap()
    ones64 = nc.dram_tensor("ones64", [128, 64], F32, kind="ExternalInput").ap()
    yout = nc.dram_tensor("yout", [T, C], F32, kind="ExternalOutput").ap()

    with tile.TileContext(nc) as tc:
        with (
            tc.tile_pool(name="const", bufs=1) as const,
            tc.tile_pool(name="persist", bufs=1) as persist,
            tc.tile_pool(name="work", bufs=2) as work,
            tc.tile_pool(name="attnp", bufs=6) as attnp,
            tc.tile_pool(name="ps", bufs=1, space="PSUM") as ps,
        ):
            # ---- constant + input loads ----------------------------------------
            # scalar queue: wqk blocks first (first matmul needs cb0), then
            # cos/sin (rope of chunk 0), wv (V of chunk 0), wproj (proj, late).
            wqk_sb = const.tile([128, CB * 512], MMD)
            for cb in range(CB):
                nc.scalar.dma_start(
                    out=wqk_sb[:, cb * 512 : (cb + 1) * 512],
                    in_=wqk[cb * 128 : (cb + 1) * 128, :],
                )
            cos_sb = const.tile([128, T], F32)
            nc.scalar.dma_start(out=cos_sb, in_=cosrep)
            sin_sb = const.tile([128, T], F32)
            nc.scalar.dma_start(out=sin_sb, in_=sinsw)
            wv_sb = const.tile([128, CB * 256], MMD)
            nc.scalar.dma_start(
                out=wv_sb.rearrange("p (cb m) -> p cb m", cb=CB),
                in_=wv.rearrange("(cb p) m -> p cb m", p=128),
            )
            wproj_sb = const.tile([128, 2 * C], MMD)
            nc.scalar.dma_start(
                out=wproj_sb.rearrange("p (cb n) -> p cb n", cb=2),
                in_=wproj.rearrange("(cb p) n -> p cb n", p=128),
            )

            # sync queue: tiny consts, xt chunk 0, masks, xt chunks 1-3.
            bqk_sb = const.tile([128, 4], F32)
            for mt in range(4):
                nc.sync.dma_start(out=bqk_sb[:, mt : mt + 1], in_=bqk[mt, :][:, None])
            ones_sb = const.tile([128, 64], F32)
            nc.sync.dma_start(out=ones_sb, in_=ones64)
            xt_sb = persist.tile([128, CB * T], MMD)   # [c128, cb, t]
            for cb in range(CB):
                nc.sync.dma_start(
                    out=xt_sb[:, cb * T : cb * T + TCH],
                    in_=xt[cb * 128 : (cb + 1) * 128, 0:TCH],
                )
            for cb in range(CB):
                nc.sync.dma_start(
                    out=xt_sb[:, cb * T + TCH : (cb + 1) * T],
                    in_=xt[cb * 128 : (cb + 1) * 128, TCH:T],
                )

            # ---- persistent intermediates --------------------------------------
            qt_sb = persist.tile([128, 2 * T], MMD)   # [Q01 | Q23], [d(2 heads), t]
            kt_sb = persist.tile([128, 2 * T], MMD)
            v_sb = persist.tile([128, NTB * 260], MMD)  # per j-block: 4x(64 v + 1 one)
            ctx0 = persist.tile([128, T], MMD)        # heads 0,1 ctxT
            ctx1 = persist.tile([128, T], MMD)        # heads 2,3 ctxT

            # ones columns of v_sb (once)
            nc.gpsimd.tensor_copy(
                v_sb.rearrange("p (tb h d) -> p tb h d", tb=NTB, h=4)[:, :, :, 64:65],
                ones_sb.rearrange("p (a b c) -> p a b c", a=NTB, b=4),
            )

            # rope swap: stream_shuffle permutes within each 32-partition
            # quadrant; host orders q/k rows [re0-15, im0-15, re16-31,
            # im16-31] per head so the rope pair-swap is i <-> 16+i.
            SWAP_MASK = list(range(16, 32)) + list(range(0, 16))
            bqk_sw = const.tile([128, 4], F32)
            nc.vector.stream_shuffle(bqk_sw, bqk_sb, SWAP_MASK)

            def emit_proj(tci):
                # output projection for chunk tci (ctx columns ready)
                for sub in range(4):
                    tb = tci * 4 + sub
                    yp = ps.tile([128, 1024], F32, tag="pa", bufs=3, name=f"yp{tb}")
                    for ncol in range(2):
                        for cbb in range(2):
                            ctx_t = ctx0 if cbb == 0 else ctx1
                            nc.tensor.matmul(
                                yp[:, ncol * 512 : (ncol + 1) * 512],
                                lhsT=ctx_t[:, tb * 128 : (tb + 1) * 128],
                                rhs=wproj_sb[:, cbb * C + ncol * 512 : cbb * C + (ncol + 1) * 512],
                                start=(cbb == 0),
                                stop=(cbb == 1),
                            )
                    ysb = work.tile([128, 1024], F32, tag="ysb", bufs=2, name=f"ysb{tb}")
                    nc.vector.tensor_copy(ysb, yp)
                    nc.sync.dma_start(out=yout[tb * 128 : (tb + 1) * 128, :], in_=ysb)

            for tci in range(NCH):
                tsl = slice(tci * TCH, (tci + 1) * TCH)

                # ---- QKV projection + rope (M-tiles: Q01 Q23 K01 K23) ----------
                for mt in range(4):
                    pq = ps.tile([128, 1024], F32, tag="pa", bufs=3, name=f"pq{tci}_{mt}")
                    for cb in range(CB):
                        nc.tensor.matmul(
                            pq[:, 0:TCH],
                            lhsT=wqk_sb[:, cb * 512 + mt * 128 : cb * 512 + (mt + 1) * 128],
                            rhs=xt_sb[:, cb * T + tci * TCH : cb * T + (tci + 1) * TCH],
                            start=(cb == 0),
                            stop=(cb == CB - 1),
                        )
                    # rope: out = (pq+b)*cos + swap(pq+b)*sin, swap via a
                    # single stream_shuffle pass instead of 4 strip-muls.
                    m1 = work.tile([128, TCH], F32, tag="m1", bufs=2, name=f"m1_{tci}_{mt}")
                    nc.vector.scalar_tensor_tensor(
                        out=m1, in0=pq[:, 0:TCH], scalar=bqk_sb[:, mt : mt + 1],
                        in1=cos_sb[:, tsl], op0=ALU.add, op1=ALU.mult,
                    )
                    shf = work.tile([128, TCH], F32, tag="shf", bufs=2, name=f"shf{tci}_{mt}")
                    nc.vector.stream_shuffle(shf, pq[:, 0:TCH], SWAP_MASK)
                    swp = work.tile([128, TCH], F32, tag="swp", bufs=2, name=f"swp{tci}_{mt}")
                    nc.vector.scalar_tensor_tensor(
                        out=swp, in0=shf, scalar=bqk_sw[:, mt : mt + 1],
                        in1=sin_sb[:, tsl], op0=ALU.add, op1=ALU.mult,
                    )
                    dest = qt_sb if mt < 2 else kt_sb
                    dcol = (mt % 2) * T + tci * TCH
                    nc.vector.tensor_add(dest[:, dcol : dcol + TCH], m1, swp)

                # ---- V projection: natural layout [t, d], 4 t-blocks -----------
                pv = ps.tile([128, 1024], F32, tag="pa", bufs=3, name=f"pv{tci}")
                for sub in range(4):
                    tb = tci * 4 + sub
                    for cb in range(CB):
                        nc.tensor.matmul(
                            pv[:, sub * 256 : (sub + 1) * 256],
                            lhsT=xt_sb[:, cb * T + tb * 128 : cb * T + (tb + 1) * 128],
                            rhs=wv_sb[:, cb * 256 : (cb + 1) * 256],
                            start=(cb == 0),
                            stop=(cb == CB - 1),
                        )
                for sub in range(4):
                    tb = tci * 4 + sub
                    vdst = v_sb[:, tb * 260 : tb * 260 + 260].rearrange(
                        "p (h d) -> p h d", h=4
                    )[:, :, 0:64]
                    vsrc = pv[:, sub * 256 : (sub + 1) * 256].rearrange(
                        "p (h d) -> p h d", h=4
                    )
                    if tci < 2:
                        nc.scalar.copy(vdst, vsrc)
                    else:
                        nc.vector.tensor_copy(vdst, vsrc)

                # ---- output projection of the previous chunk -------------------
                if tci > 0:
                    emit_proj(tci - 1)

                # ---- attention for i-chunk tci (head-pair inner) ---------------
                njb = 4 * (tci + 1)
                for pair in range(2):
                    qt_p = qt_sb[:, pair * T : (pair + 1) * T]
                    kt_p = kt_sb[:, pair * T : (pair + 1) * T]
                    ctx_p = ctx0 if pair == 0 else ctx1
                    ctxps = [
                        ps.tile([65, 512], F32, tag="ctx", bufs=2, name=f"ctxp{pair}_{tci}_{hh}")
                        for hh in range(2)
                    ]

                    def emit_av(at_pair, duo):
                        for hh in range(2):
                            for half in range(2):
                                jb = duo * 2 + half
                                h_loc = pair * 2 + hh
                                nc.tensor.matmul(
                                    ctxps[hh],
                                    lhsT=v_sb[:, jb * 260 + h_loc * 65 : jb * 260 + (h_loc + 1) * 65],
                                    rhs=at_pair[hh][:, half * 512 : (half + 1) * 512],
                                    start=(jb == 0),
                                    stop=(jb == njb - 1),
                                )

                    pending = []
                    for duo in range(njb // 2):
                        st = [
                            ps.tile([128, 1024], F32, tag="pa", bufs=3, name=f"st{pair}_{tci}_{duo}_{hh}")
                            for hh in range(2)
                        ]
                        # interleave the two heads' QK matmuls (disjoint row
                        # strips 0-63 / 64-127)
                        for half in range(2):
                            jb = duo * 2 + half
                            for hh in range(2):
                                nc.tensor.matmul(
                                    st[hh][:, half * 512 : (half + 1) * 512],
                                    lhsT=kt_p[hh * 64 : (hh + 1) * 64, jb * 128 : (jb + 1) * 128],
                                    rhs=qt_p[hh * 64 : (hh + 1) * 64, tsl],
                                    start=True,
                                    stop=True,
                                )
                        if len(pending) >= 2:
                            emit_av(*pending.pop(0))
                        at_pair = []
                        for hh in range(2):
                            at = attnp.tile([128, 1024], MMD, tag="attn", bufs=6, name=f"at{pair}_{tci}_{duo}_{hh}")
                            nc.scalar.activation(at, st[hh], AF.Exp, scale=0.125)
                            if duo >= 2 * tci:  # diagonal duo: zero j > i
                                nc.gpsimd.affine_select(
                                    out=at,
                                    in_=at,
                                    compare_op=ALU.is_ge,
                                    fill=0.0,
                                    base=tci * TCH - duo * 2 * 128,
                                    channel_multiplier=-1,
                                    pattern=[[-128, 2], [1, 512]],
                                )
                            at_pair.append(at)
                        pending.append((at_pair, duo))
                    for p in pending:
                        emit_av(*p)

                    # normalize: ctx[d, i] /= denom[i] (denom = row 64)
                    for hh in range(2):
                        ctxu = work.tile([64, 512], F32, tag="ctxu", bufs=2, name=f"cu{pair}_{tci}_{hh}")
                        nc.vector.tensor_copy(ctxu, ctxps[hh][0:64, :])
                        dn = work.tile([1, 512], F32, tag="dnrow", bufs=2, name=f"dn{pair}_{tci}_{hh}")
                        nc.vector.tensor_copy(dn, ctxps[hh][64:65, :])
                        rc = work.tile([1, 512], F32, tag="recip", bufs=2, name=f"rc{pair}_{tci}_{hh}")
                        nc.vector.reciprocal_approx_fast(out=rc, in_=dn)
                        bcast = work.tile([64, 512], F32, tag="bcast", bufs=2, name=f"bcast{pair}_{tci}_{hh}")
                        nc.gpsimd.partition_broadcast(bcast, rc)
                        nc.vector.tensor_mul(
                            ctx_p[hh * 64 : (hh + 1) * 64, tsl],
                            ctxu,
                            bcast,
                        )

            emit_proj(NCH - 1)

    nc.compile()
    return nc


def _prep_core_inputs(x, cos, sin, w_attn, b_attn, w_proj):
    """Build the 8 per-core input maps (host-side shard/reorder)."""
    import ml_dtypes
    mmnp = ml_dtypes.bfloat16
    x = np.asarray(x, dtype=np.float32)
    cos = np.asarray(cos, dtype=np.float32).reshape(T, HALF)
    sin = np.asarray(sin, dtype=np.float32).reshape(T, HALF)
    w_attn = np.asarray(w_attn, dtype=np.float32)
    b_attn = np.asarray(b_attn, dtype=np.float32)
    w_proj = np.asarray(w_proj, dtype=np.float32)

    cosT = np.ascontiguousarray(cos.T)               # [32, T]
    sinT = np.ascontiguousarray(sin.T)
    # row order within each head: [re0-15, im0-15, re16-31, im16-31] so the
    # rope pair-swap is intra-quadrant (stream_shuffle-able); sin sign is by
    # DST row (-sin for re rows, +sin for im rows).
    order16 = np.r_[0:16, 32:48, 16:32, 48:64]       # new row -> old d
    freq = order16 % 32
    sign = np.where(order16 < 32, -1.0, 1.0).astype(np.float32)
    cosrep = np.tile(cosT[freq], (2, 1))             # [128, T]
    sin_sw = np.tile(sign[:, None] * sinT[freq], (2, 1))
    ones64 = np.ones((128, 64), np.float32)

    xts = [np.ascontiguousarray(x[b].T).astype(mmnp) for b in range(B)]  # [C, T] each

    in_maps = []
    for core in range(NCORES):
        b = core // 4
        g = core % 4
        heads = [4 * g + i for i in range(NH)]
        # q/k column blocks: M-tiles [Q(h0,h1), Q(h2,h3), K(h0,h1), K(h2,h3)]
        qcols, bq = [], []
        for mt, (base, hs) in enumerate(
            [(0, heads[0:2]), (0, heads[2:4]), (C, heads[0:2]), (C, heads[2:4])]
        ):
            cols = np.concatenate([base + h * HS + order16 for h in hs])
            qcols.append(cols)
            bq.append(b_attn[cols])
        wqk_c = np.ascontiguousarray(w_attn[:, np.concatenate(qcols)]).astype(mmnp)
        bqk_c = np.stack(bq)                                            # [4, 128]
        vcols = np.concatenate(
            [np.arange(2 * C + h * HS, 2 * C + (h + 1) * HS) for h in heads]
        )
        wv_c = np.ascontiguousarray(w_attn[:, vcols]).astype(mmnp)
        wproj_c = np.ascontiguousarray(w_proj[g * 256 : (g + 1) * 256, :]).astype(mmnp)
        in_maps.append(
            {
                "xt": xts[b],
                "wqk": wqk_c,
                "wv": wv_c,
                "wproj": wproj_c,
                "bqk": np.ascontiguousarray(bqk_c),
                "cosrep": np.ascontiguousarray(cosrep),
                "sinsw": np.ascontiguousarray(sin_sw),
                "ones64": ones64,
            }
        )
    return in_maps


def kernel(x, cos, sin, w_attn, b_attn, w_proj, b_proj, _want_trace=False):
    if "nc" not in _CACHED:
        _CACHED["nc"] = _build_nc()
    nc = _CACHED["nc"]
    in_maps = _prep_core_inputs(x, cos, sin, w_attn, b_attn, w_proj)
    res = run_bass_kernel_spmd(
        nc, in_maps, core_ids=list(range(NCORES)), trace=_want_trace
    )
    _CACHED["last_result"] = res
    b_proj = np.asarray(b_proj, dtype=np.float32)
    # v-bias folds out of attention (softmax rows sum to 1): it contributes a
    # constant b_v @ w_proj to every output row, added here with b_proj.
    bv = np.asarray(b_attn, dtype=np.float32)[2 * C : 3 * C]
    bias_full = b_proj + bv @ np.asarray(w_proj, dtype=np.float32)
    out = np.empty((B, T, C), np.float32)
    for b in range(B):
        acc = res.results[b * 4]["yout"].astype(np.float32).copy()
        for g in range(1, 4):
            acc += res.results[b * 4 + g]["yout"]
        out[b] = acc + bias_full[None, :]
    return out


# revision 26
# speedup vs baseline: 1.6111x; 1.1756x over previous
"""Causal self-attention (B=2, T=2048, C=1024, H=16) on 8 TRN2 NeuronCores.

Sharding: 8 cores = 2 batches x 4 head-groups (4 heads each).
Each core computes qkv projection for its heads, attention, and a partial
output projection (its rows of w_proj); the host sums the 4 partials per
batch and adds b_proj.

v3: two phases tuned for PE contiguity (the PE clock governor needs long
uninterrupted runs, and concurrent engine activity throttles it):
  phase 1: QKV projection + rope + V projection for all chunks (PE dense,
           DVE does rope via one stream_shuffle pass, ACT evicts V).
  phase 2: per 512-token i-chunk: attention (scores -> exp -> causal mask
           -> AV with denominators from a ones-augmented V) interleaved
           with the previous chunk's output projection.
Diagonal score tiles are computed on the exact causal trapezoid: scores,
exp and AV all skip the fully-masked i-column prefix of each diagonal
j-block, and the affine_select mask runs only on the 128-column boundary
band.

Layout: all matmuls bf16 (f32 psum); x transposed [C, T] SBUF-resident;
q/k transposed [d, t] with per-head row order [re0-15, im0-15, re16-31,
im16-31] so the rope pair-swap is a single intra-quadrant stream_shuffle;
v natural [t, d]; output partials written bf16 and summed on host.
"""

import sys
import os

for _p in ("/opt/trn_rl_repo", "/root/.axon_site/_ro/trn_rl_repo"):
    if os.path.isdir(_p) and _p not in sys.path:
        sys.path.insert(0, _p)

import numpy as np
import concourse.bass as bass
import concourse.mybir as mybir
import concourse.tile as tile
from concourse import bacc
from concourse.bass_utils import run_bass_kernel_spmd

B, T, C, H = 2, 2048, 1024, 16
HS = C // H          # 64
HALF = HS // 2       # 32
NCORES = 8
NH = 4               # heads per core
TCH = 512            # chunk: t-columns per pipeline stage
NCH = T // TCH       # 4 chunks
CB = C // 128        # 8 contraction blocks
NTB = T // 128       # 16 t/j blocks
F32 = mybir.dt.float32
BF16 = mybir.dt.bfloat16
MMD = BF16
AF = mybir.ActivationFunctionType
ALU = mybir.AluOpType

_CACHED = {}


def _build_nc():
    nc = bacc.Bacc("TRN2", target_bir_lowering=False, debug=False)

    xt = nc.dram_tensor("xt", [C, T], MMD, kind="ExternalInput").ap()
    wqk = nc.dram_tensor("wqk", [C, 512], MMD, kind="ExternalInput").ap()
    wv = nc.dram_tensor("wv", [C, 256], MMD, kind="ExternalInput").ap()
    wproj = nc.dram_tensor("wproj", [256, C], MMD, kind="ExternalInput").ap()
    bqk = nc.dram_tensor("bqk", [4, 128], F32, kind="ExternalInput").ap()
    cosrep = nc.dram_tensor("cosrep", [128, T], F32, kind="ExternalInput").ap()
    sinsw = nc.dram_tensor("sinsw", [128, T], F32, kind="ExternalInput").ap()
    ones64 = nc.dram_tensor("ones64", [128, 64], F32, kind="ExternalInput").ap()
    yout = nc.dram_tensor("yout", [T, C], MMD, kind="ExternalOutput").ap()

    with tile.TileContext(nc) as tc:
        with (
            tc.tile_pool(name="const", bufs=1) as const,
            tc.tile_pool(name="persist", bufs=1) as persist,
            tc.tile_pool(name="work", bufs=2) as work,
            tc.tile_pool(name="attnp", bufs=8) as attnp,
            tc.tile_pool(name="ps", bufs=1, space="PSUM") as ps,
        ):
            # ---- constant + input loads ----------------------------------------
            # scalar queue: wqk blocks first (first matmul needs cb0), then
            # cos/sin (rope of chunk 0), wv (V of chunk 0), wproj (late).
            wqk_sb = const.tile([128, CB * 512], MMD)
            for cb in range(CB):
                nc.scalar.dma_start(
                    out=wqk_sb[:, cb * 512 : (cb + 1) * 512],
                    in_=wqk[cb * 128 : (cb + 1) * 128, :],
                )
            cos_sb = const.tile([128, T], F32)
            nc.scalar.dma_start(out=cos_sb, in_=cosrep)
            sin_sb = const.tile([128, T], F32)
            nc.scalar.dma_start(out=sin_sb, in_=sinsw)
            wv_sb = const.tile([128, CB * 256], MMD)
            nc.scalar.dma_start(
                out=wv_sb.rearrange("p (cb m) -> p cb m", cb=CB),
                in_=wv.rearrange("(cb p) m -> p cb m", p=128),
            )
            wproj_sb = const.tile([128, 2 * C], MMD)
            nc.scalar.dma_start(
                out=wproj_sb.rearrange("p (cb n) -> p cb n", cb=2),
                in_=wproj.rearrange("(cb p) n -> p cb n", p=128),
            )

            # sync queue: tiny consts, then xt chunk-by-chunk.
            bqk_sb = const.tile([128, 4], F32)
            for mt in range(4):
                nc.sync.dma_start(out=bqk_sb[:, mt : mt + 1], in_=bqk[mt, :][:, None])
            ones_sb = const.tile([128, 64], F32)
            nc.sync.dma_start(out=ones_sb, in_=ones64)
            xt_sb = persist.tile([128, CB * T], MMD)   # [c128, cb, t]
            for tci in range(NCH):
                for cb in range(CB):
                    nc.sync.dma_start(
                        out=xt_sb[:, cb * T + tci * TCH : cb * T + (tci + 1) * TCH],
                        in_=xt[cb * 128 : (cb + 1) * 128, tci * TCH : (tci + 1) * TCH],
                    )

            # ---- persistent intermediates --------------------------------------
            qt_sb = persist.tile([128, 2 * T], MMD)   # [Q01 | Q23], [d(2 heads), t]
            kt_sb = persist.tile([128, 2 * T], MMD)
            v_sb = persist.tile([128, NTB * 260], MMD)  # per j-block: 4x(64 v + 1 one)
            ctx0 = persist.tile([128, T], MMD)        # heads 0,1 ctxT
            ctx1 = persist.tile([128, T], MMD)        # heads 2,3 ctxT

            # ones columns of v_sb (once)
            nc.gpsimd.tensor_copy(
                v_sb.rearrange("p (tb h d) -> p tb h d", tb=NTB, h=4)[:, :, :, 64:65],
                ones_sb.rearrange("p (a b c) -> p a b c", a=NTB, b=4),
            )

            # rope swap: stream_shuffle permutes within each 32-partition
            # quadrant; host orders q/k rows [re0-15, im0-15, re16-31,
            # im16-31] per head so the rope pair-swap is i <-> 16+i.
            SWAP_MASK = list(range(16, 32)) + list(range(0, 16))
            bqk_sw = const.tile([128, 4], F32)
            nc.vector.stream_shuffle(bqk_sw, bqk_sb, SWAP_MASK)

            # ---- phase 1: QKV projection + rope + V, all chunks ----------------
            for tci in range(NCH):
                tsl = slice(tci * TCH, (tci + 1) * TCH)
                # M-tiles: 0=Q(h0,h1) 1=Q(h2,h3) 2=K(h0,h1) 3=K(h2,h3)
                for mt in range(4):
                    pq = ps.tile([128, 1024], F32, tag="pa", bufs=3, name=f"pq{tci}_{mt}")
                    for cb in range(CB):
                        nc.tensor.matmul(
                            pq[:, 0:TCH],
                            lhsT=wqk_sb[:, cb * 512 + mt * 128 : cb * 512 + (mt + 1) * 128],
                            rhs=xt_sb[:, cb * T + tci * TCH : cb * T + (tci + 1) * TCH],
                            start=(cb == 0),
                            stop=(cb == CB - 1),
                        )
                    # rope: out = (pq+b)*cos + swap(pq+b)*sin
                    m1 = work.tile([128, TCH], F32, tag="m1", bufs=2, name=f"m1_{tci}_{mt}")
                    nc.vector.scalar_tensor_tensor(
                        out=m1, in0=pq[:, 0:TCH], scalar=bqk_sb[:, mt : mt + 1],
                        in1=cos_sb[:, tsl], op0=ALU.add, op1=ALU.mult,
                    )
                    shf = work.tile([128, TCH], F32, tag="shf", bufs=2, name=f"shf{tci}_{mt}")
                    nc.vector.stream_shuffle(shf, pq[:, 0:TCH], SWAP_MASK)
                    swp = work.tile([128, TCH], F32, tag="swp", bufs=2, name=f"swp{tci}_{mt}")
                    nc.vector.scalar_tensor_tensor(
                        out=swp, in0=shf, scalar=bqk_sw[:, mt : mt + 1],
                        in1=sin_sb[:, tsl], op0=ALU.add, op1=ALU.mult,
                    )
                    dest = qt_sb if mt < 2 else kt_sb
                    dcol = (mt % 2) * T + tci * TCH
                    nc.vector.tensor_add(dest[:, dcol : dcol + TCH], m1, swp)

                # V projection: natural layout [t, d], 4 t-blocks per chunk
                pv = ps.tile([128, 1024], F32, tag="pa", bufs=3, name=f"pv{tci}")
                for sub in range(4):
                    tb = tci * 4 + sub
                    for cb in range(CB):
                        nc.tensor.matmul(
                            pv[:, sub * 256 : (sub + 1) * 256],
                            lhsT=xt_sb[:, cb * T + tb * 128 : cb * T + (tb + 1) * 128],
                            rhs=wv_sb[:, cb * 256 : (cb + 1) * 256],
                            start=(cb == 0),
                            stop=(cb == CB - 1),
                        )
                for sub in range(4):
                    tb = tci * 4 + sub
                    nc.scalar.copy(
                        v_sb[:, tb * 260 : tb * 260 + 260].rearrange(
                            "p (h d) -> p h d", h=4
                        )[:, :, 0:64],
                        pv[:, sub * 256 : (sub + 1) * 256].rearrange(
                            "p (h d) -> p h d", h=4
                        ),
                    )

            # ---- phase 2: attention + previous chunk's projection --------------
            def emit_proj(tci):
                for sub in range(4):
                    tb = tci * 4 + sub
                    yp = ps.tile([128, 1024], F32, tag="pa", bufs=3, name=f"yp{tb}")
                    for ncol in range(2):
                        for cbb in range(2):
                            ctx_t = ctx0 if cbb == 0 else ctx1
                            nc.tensor.matmul(
                                yp[:, ncol * 512 : (ncol + 1) * 512],
                                lhsT=ctx_t[:, tb * 128 : (tb + 1) * 128],
                                rhs=wproj_sb[:, cbb * C + ncol * 512 : cbb * C + (ncol + 1) * 512],
                                start=(cbb == 0),
                                stop=(cbb == 1),
                            )
                    ysb = work.tile([128, 1024], MMD, tag="ysb", bufs=2, name=f"ysb{tb}")
                    nc.vector.tensor_copy(ysb, yp)
                    nc.sync.dma_start(out=yout[tb * 128 : (tb + 1) * 128, :], in_=ysb)

            for tci in range(NCH):
                tsl = slice(tci * TCH, (tci + 1) * TCH)
                njb = 4 * (tci + 1)
                for pair in range(2):
                    qt_p = qt_sb[:, pair * T : (pair + 1) * T]
                    kt_p = kt_sb[:, pair * T : (pair + 1) * T]
                    ctx_p = ctx0 if pair == 0 else ctx1
                    ctxps = [
                        ps.tile([65, 512], F32, tag="ctx", bufs=2, name=f"ctxp{pair}_{tci}_{hh}")
                        for hh in range(2)
                    ]

                    def emit_av(at_pair, duo):
                        for hh in range(2):
                            for half in range(2):
                                jb = duo * 2 + half
                                q = jb - 4 * tci           # >=0 on diagonal blocks
                                c0 = max(0, q) * 128       # fully-masked i-prefix
                                h_loc = pair * 2 + hh
                                nc.tensor.matmul(
                                    ctxps[hh][:, c0:512],
                                    lhsT=v_sb[:, jb * 260 + h_loc * 65 : jb * 260 + (h_loc + 1) * 65],
                                    rhs=at_pair[hh][:, half * 512 + c0 : (half + 1) * 512],
                                    start=(jb == 0),
                                    stop=(jb == njb - 1),
                                )

                    pending = []
                    for duo in range(njb // 2):
                        st = [
                            ps.tile([128, 1024], F32, tag="pa", bufs=3, name=f"st{pair}_{tci}_{duo}_{hh}")
                            for hh in range(2)
                        ]
                        # interleave the two heads' QK matmuls (disjoint row
                        # strips 0-63 / 64-127)
                        for half in range(2):
                            jb = duo * 2 + half
                            q = jb - 4 * tci
                            c0 = max(0, q) * 128
                            for hh in range(2):
                                nc.tensor.matmul(
                                    st[hh][:, half * 512 + c0 : (half + 1) * 512],
                                    lhsT=kt_p[hh * 64 : (hh + 1) * 64, jb * 128 : (jb + 1) * 128],
                                    rhs=qt_p[hh * 64 : (hh + 1) * 64, tci * TCH + c0 : (tci + 1) * TCH],
                                    start=True,
                                    stop=True,
                                )
                        if len(pending) >= 3:
                            emit_av(*pending.pop(0))
                        at_pair = []
                        for hh in range(2):
                            at = attnp.tile([128, 1024], MMD, tag="attn", bufs=8, name=f"at{pair}_{tci}_{duo}_{hh}")
                            if duo >= 2 * tci:
                                # diagonal duo: exp the kept column range of
                                # each half, then mask the boundary band.
                                for half in range(2):
                                    q = duo * 2 + half - 4 * tci
                                    c0 = q * 128
                                    nc.scalar.activation(
                                        at[:, half * 512 + c0 : (half + 1) * 512],
                                        st[hh][:, half * 512 + c0 : (half + 1) * 512],
                                        AF.Exp, scale=0.125,
                                    )
                                    nc.gpsimd.affine_select(
                                        out=at[:, half * 512 + c0 : half * 512 + c0 + 128],
                                        in_=at[:, half * 512 + c0 : half * 512 + c0 + 128],
                                        compare_op=ALU.is_ge,
                                        fill=0.0,
                                        base=0,
                                        channel_multiplier=-1,
                                        pattern=[[1, 128]],
                                    )
                            else:
                                nc.scalar.activation(at, st[hh], AF.Exp, scale=0.125)
                            at_pair.append(at)
                        pending.append((at_pair, duo))
                    for p in pending:
                        emit_av(*p)

                    # normalize: ctx[d, i] /= denom[i] (denom = row 64); recip
                    # and the final mul read the ctx psum directly.
                    for hh in range(2):
                        dn = work.tile([1, 512], F32, tag="dnrow", bufs=2, name=f"dn{pair}_{tci}_{hh}")
                        nc.vector.tensor_copy(dn, ctxps[hh][64:65, :])
                        rc = work.tile([1, 512], F32, tag="recip", bufs=2, name=f"rc{pair}_{tci}_{hh}")
                        nc.vector.reciprocal_approx_fast(out=rc, in_=dn)
                        bcast = work.tile([64, 512], F32, tag="bcast", bufs=2, name=f"bcast{pair}_{tci}_{hh}")
                        nc.gpsimd.partition_broadcast(bcast, rc)
                        nc.vector.tensor_mul(
                            ctx_p[hh * 64 : (hh + 1) * 64, tsl],
                            ctxps[hh][0:64, :],
                            bcast,
                        )

                    if pair == 0 and tci > 0:
                        emit_proj(tci - 1)

            emit_proj(NCH - 1)

    nc.compile()
    return nc


def _prep_core_inputs(x, cos, sin, w_attn, b_attn, w_proj):
    """Build the 8 per-core input maps (host-side shard/reorder)."""
    import ml_dtypes
    mmnp = ml_dtypes.bfloat16
    x = np.asarray(x, dtype=np.float32)
    cos = np.asarray(cos, dtype=np.float32).reshape(T, HALF)
    sin = np.asarray(sin, dtype=np.float32).reshape(T, HALF)
    w_attn = np.asarray(w_attn, dtype=np.float32)
    b_attn = np.asarray(b_attn, dtype=np.float32)
    w_proj = np.asarray(w_proj, dtype=np.float32)

    cosT = np.ascontiguousarray(cos.T)               # [32, T]
    sinT = np.ascontiguousarray(sin.T)
    # row order within each head: [re0-15, im0-15, re16-31, im16-31] so the
    # rope pair-swap is intra-quadrant (stream_shuffle-able); sin sign is by
    # DST row (-sin for re rows, +sin for im rows).
    order16 = np.r_[0:16, 32:48, 16:32, 48:64]       # new row -> old d
    freq = order16 % 32
    sign = np.where(order16 < 32, -1.0, 1.0).astype(np.float32)
    cosrep = np.tile(cosT[freq], (2, 1))             # [128, T]
    sin_sw = np.tile(sign[:, None] * sinT[freq], (2, 1))
    ones64 = np.ones((128, 64), np.float32)

    xts = [np.ascontiguousarray(x[b].T).astype(mmnp) for b in range(B)]  # [C, T] each

    in_maps = []
    for core in range(NCORES):
        b = core // 4
        g = core % 4
        heads = [4 * g + i for i in range(NH)]
        # q/k column blocks: M-tiles [Q(h0,h1), Q(h2,h3), K(h0,h1), K(h2,h3)]
        qcols, bq = [], []
        for mt, (base, hs) in enumerate(
            [(0, heads[0:2]), (0, heads[2:4]), (C, heads[0:2]), (C, heads[2:4])]
        ):
            cols = np.concatenate([base + h * HS + order16 for h in hs])
            qcols.append(cols)
            bq.append(b_attn[cols])
        wqk_c = np.ascontiguousarray(w_attn[:, np.concatenate(qcols)]).astype(mmnp)
        bqk_c = np.stack(bq)                                            # [4, 128]
        vcols = np.concatenate(
            [np.arange(2 * C + h * HS, 2 * C + (h + 1) * HS) for h in heads]
        )
        wv_c = np.ascontiguousarray(w_attn[:, vcols]).astype(mmnp)
        wproj_c = np.ascontiguousarray(w_proj[g * 256 : (g + 1) * 256, :]).astype(mmnp)
        in_maps.append(
            {
                "xt": xts[b],
                "wqk": wqk_c,
                "wv": wv_c,
                "wproj": wproj_c,
                "bqk": np.ascontiguousarray(bqk_c),
                "cosrep": np.ascontiguousarray(cosrep),
                "sinsw": np.ascontiguousarray(sin_sw),
                "ones64": ones64,
            }
        )
    return in_maps


def kernel(x, cos, sin, w_attn, b_attn, w_proj, b_proj, _want_trace=False):
    if "nc" not in _CACHED:
        _CACHED["nc"] = _build_nc()
    nc = _CACHED["nc"]
    in_maps = _prep_core_inputs(x, cos, sin, w_attn, b_attn, w_proj)
    res = run_bass_kernel_spmd(
        nc, in_maps, core_ids=list(range(NCORES)), trace=_want_trace
    )
    _CACHED["last_result"] = res
    b_proj = np.asarray(b_proj, dtype=np.float32)
    # v-bias folds out of attention (softmax rows sum to 1): it contributes a
    # constant b_v @ w_proj to every output row, added here with b_proj.
    bv = np.asarray(b_attn, dtype=np.float32)[2 * C : 3 * C]
    bias_full = b_proj + bv @ np.asarray(w_proj, dtype=np.float32)
    out = np.empty((B, T, C), np.float32)
    for b in range(B):
        acc = res.results[b * 4]["yout"].astype(np.float32).copy()
        for g in range(1, 4):
            acc += res.results[b * 4 + g]["yout"]
        out[b] = acc + bias_full[None, :]
    return out
